# revision 12
# baseline (speedup 1.0000x reference)
"""Trainium2 Bass kernel for nn_IdentityConvolution.

reference semantics:
    r = sum_c x_real[b, c, :, :]   # [B, 1, H, W]
    i = sum_c x_imag[b, c, :, :]
    out = complex(r, i) broadcast to [B, 64, H, W]  (complex64)

Sharding: data-parallel over batch B=8 across the 8 NeuronCores (one
batch image per core, no cross-core communication).

Per-core device program (Tile-scheduled), built to minimize the busiest
compute engine (the harness-metric bottleneck) by splitting the channel
reduction across the Pool (GpSimd) and DVE engines in fp16:

  - inputs viewed as [C=64, P=128, Q=512] (hw = p*512 + q), processed in
    nhw=2 q-chunks of 256.
  - per chunk and lane (real/imag): 4 input tiles [128, 16, qc] f32 are
    DMA'd on the SP/Activation HWDGE queues; a level-1 add folds each
    tile's 16 channels to 8 in one op writing fp16 into a shared wide
    scratch [128, 32, qc] (Pool does 7 of the 8 level-1 adds per chunk,
    DVE does 1 — balances engine busy ~23us each).
  - DVE then runs one merged reduction chain 32->16->8->4->2 in fp16
    (2 elem/cycle/lane mode) and the final add writes the complex-
    interleaved f32 output tile directly ([128, q, 2] strided view).
  - the [128, 2*qc] f32 tile is broadcast-DMA'd to all 64 output channel
    planes (8 dma_starts of 8 planes each, stride-0 source AP) on the
    SP/Act queues.

fp16 intermediate precision: inputs are ~N(0,1), channel sums |.| < ~40;
tree rounding gives rel err ~6e-4 vs the 2e-2 gate.
"""

import sys

sys.path.insert(0, "/opt/trn_rl_repo")

from contextlib import ExitStack

import numpy as np

import concourse.bacc as bacc
import concourse.tile as tile
from concourse import mybir
from concourse.bass_utils import run_bass_kernel_spmd

B, C, H, W = 8, 64, 256, 256
P = 128
Q = (H * W) // P  # 512
NHW = 2  # q chunks
QC = Q // NHW  # 256

F32 = mybir.dt.float32
F16 = mybir.dt.float16

_cache = {}


def _build_program(
    repeat=1,
    barrier=False,
    nhw=NHW,
    dve_l1=1,  # level-1 adds per chunk on DVE (of 8); rest on Pool
    split_l1=4,  # split one L1 add per rep: this many channel-pairs on DVE
    out_bcast=8,
    in_q="sasasasa",
    out_q="asasasas",
    inbufs=8,
    scrbufs=2,
):
    qc = Q // nhw
    nc = bacc.Bacc("TRN2", target_bir_lowering=False, debug=False, num_devices=8)
    xr = nc.dram_tensor("x_real", [C, P, Q], F32, kind="ExternalInput").ap()
    xi = nc.dram_tensor("x_imag", [C, P, Q], F32, kind="ExternalInput").ap()
    out = nc.dram_tensor("out", [C, P, 2 * Q], F32, kind="ExternalOutput").ap()
    xr_v = xr.rearrange("c p q -> p c q")
    xi_v = xi.rearrange("c p q -> p c q")
    emap = {"s": nc.sync, "a": nc.scalar, "g": nc.gpsimd, "v": nc.vector}

    with tile.TileContext(nc) as tc, ExitStack() as ctx, nc.allow_low_precision(
        "channel-sum of ~N(0,1) fits fp16; harness tolerance 2e-2"
    ):
        inp = ctx.enter_context(tc.tile_pool(name="inp", bufs=inbufs))
        scr = ctx.enter_context(tc.tile_pool(name="scr", bufs=scrbufs))
        accp = ctx.enter_context(tc.tile_pool(name="acc", bufs=2))
        outp = ctx.enter_context(tc.tile_pool(name="outp", bufs=2))
        for r in range(repeat):
            if r and barrier:
                tc.strict_bb_all_engine_barrier()
            for j in range(nhw):
                q0 = j * qc
                ot = outp.tile([P, 2 * qc], F32, tag="ot")
                otv = ot[:].rearrange("p (q t) -> p q t", t=2)
                k = 0
                for lane, x_v in enumerate((xr_v, xi_v)):
                    s = scr.tile([P, 32, qc], F16, tag=f"s{lane}")
                    for g in range(4):
                        t = inp.tile([P, 16, qc], F32, tag="in")
                        emap[in_q[(lane * 4 + g) % len(in_q)]].dma_start(
                            out=t[:],
                            in_=x_v[:, g * 16 : (g + 1) * 16, q0 : q0 + qc],
                        )
                        if k == dve_l1 and j == 0 and split_l1:
                            # fine-grain balance: split this L1 h/(8-h)
                            h = split_l1
                            nc.vector.tensor_add(
                                s[:, g * 8 : g * 8 + h, :],
                                t[:, 0:h, :],
                                t[:, 8 : 8 + h, :],
                            )
                            nc.gpsimd.tensor_add(
                                s[:, g * 8 + h : g * 8 + 8, :],
                                t[:, h:8, :],
                                t[:, 8 + h : 16, :],
                            )
                        else:
                            l1 = nc.vector if k < dve_l1 else nc.gpsimd
                            l1.tensor_add(
                                s[:, g * 8 : g * 8 + 8, :],
                                t[:, 0:8, :],
                                t[:, 8:16, :],
                            )
                        k += 1
                    nc.vector.tensor_add(
                        s[:, 0:16, :], s[:, 0:16, :], s[:, 16:32, :]
                    )
                    nc.vector.tensor_add(s[:, 0:8, :], s[:, 0:8, :], s[:, 8:16, :])
                    nc.vector.tensor_add(s[:, 0:4, :], s[:, 0:4, :], s[:, 4:8, :])
                    nc.vector.tensor_add(s[:, 0:2, :], s[:, 0:2, :], s[:, 2:4, :])
                    # final level in fp16 (2x mode), then the idle Act engine
                    # does the cast + complex-interleave write
                    acc = accp.tile([P, qc], F16, tag=f"acc{lane}")
                    nc.vector.tensor_add(acc[:], s[:, 0, :], s[:, 1, :])
                    nc.scalar.activation(
                        otv[:, :, lane],
                        acc[:],
                        mybir.ActivationFunctionType.Copy,
                    )
                for m, co in enumerate(range(0, C, out_bcast)):
                    eng = emap[out_q[m % len(out_q)]]
                    eng.dma_start(
                        out=out[co : co + out_bcast, :, 2 * q0 : 2 * q0 + 2 * qc]
                        .rearrange("c p q -> p c q"),
                        in_=ot[:].unsqueeze(1).broadcast_to((P, out_bcast, 2 * qc)),
                    )
    nc.compile()
    return nc


def kernel(x_real, x_imag, _profile=False):
    if "nc" not in _cache:
        _cache["nc"] = _build_program()
    nc = _cache["nc"]

    x_real = np.asarray(x_real)
    x_imag = np.asarray(x_imag)
    in_maps = [
        {
            "x_real": np.ascontiguousarray(x_real[b]).reshape(C, P, Q),
            "x_imag": np.ascontiguousarray(x_imag[b]).reshape(C, P, Q),
        }
        for b in range(B)
    ]
    res = run_bass_kernel_spmd(nc, in_maps, list(range(B)), trace=_profile)
    _cache["last_result"] = res

    out = np.empty((B, C, H, W), dtype=np.complex64)
    for b in range(B):
        o = res.results[b]["out"]  # [C, P, 2Q] f32
        out[b] = o.reshape(C, P * Q, 2).view(np.complex64).reshape(C, H, W)
    return out


# revision 14
# speedup vs baseline: 1.2112x; 1.2112x over previous
"""Trainium2 Bass kernel for nn_IdentityConvolution.

reference semantics:
    r = sum_c x_real[b, c, :, :]   # [B, 1, H, W]
    i = sum_c x_imag[b, c, :, :]
    out = complex(r, i) broadcast to [B, 64, H, W]  (complex64)

Sharding: data-parallel over batch B=8 across the 8 NeuronCores (one
batch image per core, no cross-core communication).

Per-core device program (Tile-scheduled), built to minimize the busiest
compute engine (the harness-metric bottleneck) by splitting the channel
reduction across the Pool (GpSimd) and DVE engines in fp16:

  - inputs viewed as [C=64, P=128, Q=512] (hw = p*512 + q), processed in
    nhw=2 q-chunks of 256.
  - per chunk and lane (real/imag): 4 input tiles [128, 16, qc] f32 are
    DMA'd on the SP/Activation HWDGE queues; a level-1 add folds each
    tile's 16 channels to 8 in one op writing fp16 into a shared wide
    scratch [128, 32, qc] (Pool does 7 of the 8 level-1 adds per chunk,
    DVE does 1 — balances engine busy ~23us each).
  - DVE then runs one merged reduction chain 32->16->8->4->2 in fp16
    (2 elem/cycle/lane mode) and the final add writes the complex-
    interleaved f32 output tile directly ([128, q, 2] strided view).
  - the [128, 2*qc] f32 tile is broadcast-DMA'd to all 64 output channel
    planes (8 dma_starts of 8 planes each, stride-0 source AP) on the
    SP/Act queues.

fp16 intermediate precision: inputs are ~N(0,1), channel sums |.| < ~40;
tree rounding gives rel err ~6e-4 vs the 2e-2 gate.
"""

import sys

sys.path.insert(0, "/opt/trn_rl_repo")

from contextlib import ExitStack

import numpy as np

import concourse.bacc as bacc
import concourse.tile as tile
from concourse import mybir
from concourse.bass_utils import run_bass_kernel_spmd

B, C, H, W = 8, 64, 256, 256
P = 128
Q = (H * W) // P  # 512
NHW = 2  # q chunks
QC = Q // NHW  # 256

F32 = mybir.dt.float32
F16 = mybir.dt.float16

_cache = {}


def _build_program(
    repeat=1,
    barrier=False,
    nhw=NHW,
    dve_l1=2,  # level-1 adds per chunk on DVE (of 8); rest on Pool
    split_l1=6,  # split one L1 add per rep: this many channel-pairs on DVE
    out_bcast=8,
    in_q="sasasasa",
    out_q="asasasas",
    inbufs=8,
    scrbufs=2,
):
    qc = Q // nhw
    nc = bacc.Bacc("TRN2", target_bir_lowering=False, debug=False, num_devices=8)
    xr = nc.dram_tensor("x_real", [C, P, Q], F32, kind="ExternalInput").ap()
    xi = nc.dram_tensor("x_imag", [C, P, Q], F32, kind="ExternalInput").ap()
    out = nc.dram_tensor("out", [C, P, 2 * Q], F32, kind="ExternalOutput").ap()
    xr_v = xr.rearrange("c p q -> p c q")
    xi_v = xi.rearrange("c p q -> p c q")
    emap = {"s": nc.sync, "a": nc.scalar, "g": nc.gpsimd, "v": nc.vector}

    with tile.TileContext(nc) as tc, ExitStack() as ctx, nc.allow_low_precision(
        "channel-sum of ~N(0,1) fits fp16; harness tolerance 2e-2"
    ):
        inp = ctx.enter_context(tc.tile_pool(name="inp", bufs=inbufs))
        scr = ctx.enter_context(tc.tile_pool(name="scr", bufs=scrbufs))
        accp = ctx.enter_context(tc.tile_pool(name="acc", bufs=2))
        outp = ctx.enter_context(tc.tile_pool(name="outp", bufs=2))
        for r in range(repeat):
            if r and barrier:
                tc.strict_bb_all_engine_barrier()
            for j in range(nhw):
                q0 = j * qc
                ot = outp.tile([P, 2 * qc], F32, tag="ot")
                otv = ot[:].rearrange("p (q t) -> p q t", t=2)
                s = scr.tile([P, 64, qc], F16, tag="s")
                k = 0
                for lane, x_v in enumerate((xr_v, xi_v)):
                    for g in range(4):
                        t = inp.tile([P, 16, qc], F32, tag="in")
                        emap[in_q[(lane * 4 + g) % len(in_q)]].dma_start(
                            out=t[:],
                            in_=x_v[:, g * 16 : (g + 1) * 16, q0 : q0 + qc],
                        )
                        c0 = lane * 32 + g * 8
                        if k == dve_l1 and j == 0 and split_l1:
                            # fine-grain balance: split this L1 h/(8-h)
                            h = split_l1
                            nc.vector.tensor_add(
                                s[:, c0 : c0 + h, :],
                                t[:, 0:h, :],
                                t[:, 8 : 8 + h, :],
                            )
                            nc.gpsimd.tensor_add(
                                s[:, c0 + h : c0 + 8, :],
                                t[:, h:8, :],
                                t[:, 8 + h : 16, :],
                            )
                        else:
                            l1 = nc.vector if k < dve_l1 else nc.gpsimd
                            l1.tensor_add(
                                s[:, c0 : c0 + 8, :],
                                t[:, 0:8, :],
                                t[:, 8:16, :],
                            )
                        k += 1
                # merged deep chain: each level is one DVE op covering both
                # lanes (4D AP) — runs in a faster DVE mode than per-lane 3D
                v = s[:].rearrange("p (l c) q -> p l c q", l=2)
                nc.vector.tensor_add(
                    v[:, :, 0:16, :], v[:, :, 0:16, :], v[:, :, 16:32, :]
                )
                nc.vector.tensor_add(
                    v[:, :, 0:8, :], v[:, :, 0:8, :], v[:, :, 8:16, :]
                )
                nc.vector.tensor_add(
                    v[:, :, 0:4, :], v[:, :, 0:4, :], v[:, :, 4:8, :]
                )
                nc.vector.tensor_add(
                    v[:, :, 0:2, :], v[:, :, 0:2, :], v[:, :, 2:4, :]
                )
                acc2 = accp.tile([P, 2, qc], F16, tag="acc")
                nc.vector.tensor_add(acc2[:], v[:, :, 0, :], v[:, :, 1, :])
                # idle Act engine casts fp16->f32 into the complex-interleaved
                # output (per-lane 3D out APs; a rearranged 4D out AP here
                # crashes the device)
                nc.scalar.activation(
                    otv[:, :, 0], acc2[:, 0, :], mybir.ActivationFunctionType.Copy
                )
                nc.scalar.activation(
                    otv[:, :, 1], acc2[:, 1, :], mybir.ActivationFunctionType.Copy
                )
                for m, co in enumerate(range(0, C, out_bcast)):
                    eng = emap[out_q[m % len(out_q)]]
                    eng.dma_start(
                        out=out[co : co + out_bcast, :, 2 * q0 : 2 * q0 + 2 * qc]
                        .rearrange("c p q -> p c q"),
                        in_=ot[:].unsqueeze(1).broadcast_to((P, out_bcast, 2 * qc)),
                    )
    nc.compile()
    return nc


def kernel(x_real, x_imag, _profile=False):
    if "nc" not in _cache:
        _cache["nc"] = _build_program()
    nc = _cache["nc"]

    x_real = np.asarray(x_real)
    x_imag = np.asarray(x_imag)
    in_maps = [
        {
            "x_real": np.ascontiguousarray(x_real[b]).reshape(C, P, Q),
            "x_imag": np.ascontiguousarray(x_imag[b]).reshape(C, P, Q),
        }
        for b in range(B)
    ]
    res = run_bass_kernel_spmd(nc, in_maps, list(range(B)), trace=_profile)
    _cache["last_result"] = res

    out = np.empty((B, C, H, W), dtype=np.complex64)
    for b in range(B):
        o = res.results[b]["out"]  # [C, P, 2Q] f32
        out[b] = o.reshape(C, P * Q, 2).view(np.complex64).reshape(C, H, W)
    return out
